# revision 1
# baseline (speedup 1.0000x reference)
"""BTV loss kernel for Trainium2 (8 NeuronCores, Bass/Tile).

reference: total = sum over 7x7 neighborhood shifts (k,l) != (0,0) of
           sqrt((x - roll(x,(k,l),axis=(2,3)))**2 + 1e-6).sum()
           out = 0.1 * total / x.size

Math used here:
  - circular-shift symmetry: shift (k,l) and (-k,-l) give identical sums,
    so only the 24 half-space shifts {k>0, any l} u {k==0, l>0} are
    computed and the result doubled.
  - sqrt(d^2 + 1e-6) ~= |d|: relative error of the final sum ~3e-6
    (verified numerically in f64), far below tolerance.
  - bf16 differences: |d| in bf16 adds ~1e-5 relative error (verified).

Pipeline per 128-row block (per core: 3 images x 8 blocks):
  - one DMA loads rows [128r, 128r+131) of all 3 images in bf16, twice
    (even + odd column phase) so every shifted view is 4B-aligned and
    DVE tensor ops run in 2x/4x packed modes.
  - DVE tensor_tensor subtract (bf16, 2x) per shift
  - |d| + free-dim reduce: split between ACT (activation Abs with
    accum_out, 1x but otherwise idle) and DVE (tensor_scalar abs_max 0
    with accum_out, 4x) to balance engine busy time.
  - per-partition partials accumulate in a (128, 192) f32 stage,
    reduced once at the end; host sums 8x128 values in f64.

Distribution: pure data parallel over the 24 (b,c) images, 3 per core.
"""

import dataclasses
import re
from operator import add as _py_add

import numpy as np

import concourse.bass as bass
import concourse.bacc as bacc_mod
import concourse.mybir as mybir
from concourse import dve_ops as _dvo
from concourse.dve_spec import AluOp as _DveAluOp
from concourse.dve_spec import Bin, Spec, Src0, Src1
from concourse.tile import TileContext
from concourse.bass_utils import run_bass_kernel_spmd

B, C, H, W = 8, 3, 1024, 1024
NCORES = 8
IMGS = (B * C) // NCORES        # images per core = 3
BASE = 4                        # left col pad (even => 4B-aligned in bf16)
WP = W + BASE + 3 + 1           # 1032: [w-4..w-1][0..1023][0,1,2][pad]
RB = 128                        # rows per block (partition dim)
NBLK = H // RB                  # 8 row blocks per image
ROWS_BLK = RB + 3               # 131 rows stored per block (128 + 3 halo)
# half-space shifts: (k>0, any l) or (k==0, l>0)
SHIFTS = [(k, l) for k in range(0, 4) for l in range(-3, 4) if (k > 0 or l > 0)]
assert len(SHIFTS) == 24
# which shifts run fully on DVE via the fused custom op ABS_DIFF_REDUCE
# (|a-b| + free-dim sum in one 1x instruction, ~3327ns) vs the split path
# (DVE bf16 2x subtract ~1669ns + ACT Abs/accum, batched 4 shifts per
# activation instruction to amortize the ~420ns fixed overhead).
FUSED = {2, 6, 10, 14, 18, 22}
ACT_GROUP = 4

WEIGHT = 0.1
F32 = mybir.dt.float32
BF16 = mybir.dt.bfloat16

_OPS_CACHE = None


def _mk_absdiff_uop(two_x: bool, kind: str):
    """One UopConfig for the absdiff-accumulate family.

    kind: "seed"   - first element(-pair) loads the accumulator (blk3)
          "steady" - accumulates into blk3's CURR_ALU_OUT flop
    1x datapath: blk0 |d|=ABSDIFF(lane0, chain0=SRC_1); blk1 captures |d|
    onto chain0, ALU bypasses; blk2 bypass; blk3 acc; blk4-7 bypass;
    WR0_LO <- DELAY_0.
    2x datapath: blk0 |d_lo|, blk1 |d_hi| (chains 1/2 = HI inputs),
    blk2 pair-sum (chain0=|d_lo|, captures |d_hi| on chain1); blk3 acc;
    WR0_LO <- DELAY_0, WR0_HI <- DELAY_1.
    """
    from concourse.dve_uop import (
        ENABLE,
        AluInp,
        DelayInp,
        InpSel,
        OutPath,
        OutSel,
        Trigger,
        UopConfig,
    )
    from concourse.dve_uop import AluOp as UAluOp

    seed = kind == "seed"
    u = UopConfig()
    u.enable_input(InpSel.SRC_0, 0)
    u.enable_input(InpSel.SRC_1, 1)
    if two_x:
        u.enable_input(InpSel.SRC_0_HI, 2)
        u.enable_input(InpSel.SRC_1_HI, 3)
    u.accum_enabled = ENABLE
    dp = u.datapath_config
    dp[0].enable_alu(UAluOp.ABSOLUTE_DIFF, AluInp.PREV_ALU_OUT, AluInp.PREV_DELAY_0)
    if two_x:
        dp[0].pass_through_delay(1, 2)
        dp[1].enable_alu(
            UAluOp.ABSOLUTE_DIFF, AluInp.PREV_DELAY_1, AluInp.PREV_DELAY_2
        )
        dp[1].enable_delay_from_src(DelayInp.PREV_ALU_OUT, 0)
        dp[2].enable_alu(UAluOp.ADD, AluInp.PREV_ALU_OUT, AluInp.PREV_DELAY_0)
        dp[2].enable_delay_from_src(DelayInp.PREV_ALU_OUT, 1)
        dp[2].pass_through_delay(0)
    else:
        dp[1].enable_alu(UAluOp.BYPASS, AluInp.PREV_ALU_OUT, AluInp.PREV_ALU_OUT)
        dp[1].enable_delay_from_src(DelayInp.PREV_ALU_OUT, 0)
        dp[2].enable_alu(UAluOp.BYPASS, AluInp.PREV_ALU_OUT, AluInp.PREV_ALU_OUT)
        dp[2].pass_through_delay(0)
    if seed:
        dp[3].enable_alu(UAluOp.BYPASS, AluInp.PREV_ALU_OUT, AluInp.PREV_ALU_OUT)
    else:
        dp[3].enable_alu(UAluOp.ADD, AluInp.CURR_ALU_OUT, AluInp.PREV_ALU_OUT)
    dp[3].pass_through_delay(0, 1) if two_x else dp[3].pass_through_delay(0)
    dp[3].alu_out_a_enable = ENABLE
    for b in (4, 5, 6, 7):
        dp[b].enable_alu(UAluOp.BYPASS, AluInp.PREV_ALU_OUT, AluInp.PREV_ALU_OUT)
        dp[b].pass_through_delay(0, 1) if two_x else dp[b].pass_through_delay(0)
        dp[b].alu_out_a_enable = ENABLE
    u.require_inp0 = ENABLE
    u.require_inp1 = ENABLE
    u.enable_output(OutSel.DELAY_0, OutPath.WR0_LO)
    if two_x:
        u.enable_output(OutSel.DELAY_1, OutPath.WR0_HI)
    if seed:
        u.trigger = (Trigger.COUNT, Trigger.SRC_TENSOR_DONE, Trigger.NONE)
        u.next_uop = (1, 0, 0)
        u.repeat_count = 1
    else:
        u.trigger = (Trigger.SRC_TENSOR_DONE, Trigger.NONE, Trigger.NONE)
        u.next_uop = (0, 0, 0)
    return u


def _mk_read_uop():
    """Single 1-element uop: route blk3's persistent accumulator flop to
    the output (blk3 BYPASS(CURR_ALU_OUT) -> blk4-7 bypass -> WR0_LO)."""
    from concourse.dve_uop import (
        ENABLE,
        AluInp,
        InpSel,
        OutPath,
        OutSel,
        Trigger,
        UopConfig,
    )
    from concourse.dve_uop import AluOp as UAluOp

    u = UopConfig()
    u.enable_input(InpSel.SRC_0, 0)
    dp = u.datapath_config
    for b in (0, 1, 2):
        dp[b].enable_alu(UAluOp.BYPASS, AluInp.PREV_ALU_OUT, AluInp.PREV_ALU_OUT)
    dp[3].enable_alu(UAluOp.BYPASS, AluInp.CURR_ALU_OUT, AluInp.CURR_ALU_OUT)
    for b in (4, 5, 6, 7):
        dp[b].enable_alu(UAluOp.BYPASS, AluInp.PREV_ALU_OUT, AluInp.PREV_ALU_OUT)
    u.require_inp0 = ENABLE
    u.enable_output(OutSel.ALU_OUT, OutPath.WR0_LO)
    u.trigger = (Trigger.SRC_TENSOR_DONE, Trigger.NONE, Trigger.NONE)
    u.next_uop = (0, 0, 0)
    return u


class _HandDveOp(_dvo.DveOp):
    """DveOp with hand-authored uop programs (1x and optional 2x)."""

    BUILDERS = {}  # name -> (build_1x, build_2x_or_None, rd1_en)

    def compile(self, ver):
        from concourse.dve_uop import DveOpSpec

        key = (self.name, ver)
        if (r := _dvo._COMPILE_CACHE.get(key)) is not None:
            return r
        b1, b2, rd1 = self.BUILDERS[self.name]
        result = DveOpSpec(
            name=self.name,
            opcode=_dvo.get_dve_sub_opcode(self.name),
            uops=b1(),
            uops_2x=(b2() if b2 is not None else None),
            rd1_en=rd1,
        )
        got = result.sha(ver)
        if self.uops_sha.get(ver) != got:
            raise ValueError(f"sha drift ({ver}: {got} != pinned)")
        _dvo._COMPILE_CACHE[key] = result
        return result


def _register(name, spec, build_1x, build_2x, rd1_en):
    _HandDveOp.BUILDERS[name] = (build_1x, build_2x, rd1_en)
    op = _HandDveOp(name, spec, subdim=False, uops_sha={})
    _dvo._SUB_OPCODE_FOR_NAME[name] = _dvo._CUSTOM_DVE_ROW_BASE + len(_dvo.OPS)
    shas = {}
    for ver in ("v3", "v4"):
        try:
            op.compile(ver)
            shas[ver] = op.uops_sha.get(ver)
        except ValueError as e:
            m = re.search(r"([0-9a-f]{16})", str(e))
            if not m:
                raise
            shas[ver] = m.group(1)
    op = dataclasses.replace(op, uops_sha=shas)
    _dvo.OPS.append(op)
    _dvo.CUSTOM_DVE_SPECS[name] = spec
    return op


def _get_ops():
    """Register (once per process) the three custom DVE ops and return
    (seed, cont, read)."""
    global _OPS_CACHE
    if _OPS_CACHE is not None:
        return _OPS_CACHE
    have = {op.name: op for op in _dvo.OPS}
    if "ABSDIFF_ACC_SEED_ANT" in have:
        _OPS_CACHE = (
            have["ABSDIFF_ACC_SEED_ANT"],
            have["ABSDIFF_ACC_CONT_ANT"],
            have["ABSDIFF_ACC_READ_ANT"],
        )
        return _OPS_CACHE

    def _ref_acc(in0, in1, s0, s1, imm2):
        b = np.abs(in0.astype(np.float32) - in1.astype(np.float32)).astype(
            np.float32
        )
        return b, b.reshape(b.shape[0], -1).sum(axis=-1, keepdims=True)

    spec_acc = Spec(
        body=Bin(_DveAluOp.ABSOLUTE_DIFF, Src0, Src1),
        accum=_py_add,
        reference=_ref_acc,
    )
    spec_read = Spec(
        body=Src0,
        reference=lambda in0, in1, s0, s1, imm2: in0.astype(np.float32),
    )
    seed = _register(
        "ABSDIFF_ACC_SEED_ANT",
        spec_acc,
        lambda: [_mk_absdiff_uop(False, "seed"), _mk_absdiff_uop(False, "steady")],
        lambda: [_mk_absdiff_uop(True, "seed"), _mk_absdiff_uop(True, "steady")],
        True,
    )
    cont = _register(
        "ABSDIFF_ACC_CONT_ANT",
        spec_acc,
        lambda: [_mk_absdiff_uop(False, "steady")],
        lambda: [_mk_absdiff_uop(True, "steady")],
        True,
    )
    read = _register(
        "ABSDIFF_ACC_READ_ANT",
        spec_read,
        lambda: [_mk_read_uop()],
        None,
        False,
    )
    _OPS_CACHE = (seed, cont, read)
    return _OPS_CACHE


CHAIN = 8  # fused ops per accumulator chain (one READ per chain)


def _build_nc():
    seed_op, cont_op, read_op = _get_ops()
    nc = bacc_mod.Bacc("TRN2", target_bir_lowering=False)
    # host layout: x[r, q, j, i, c] = pad_j[i, 128*r + q, c]; j=0 even
    # phase, j=1 odd phase (odd[c] = even[c+1]).
    X = nc.dram_tensor(
        "x", [NBLK, ROWS_BLK, IMGS, WP], BF16, kind="ExternalInput"
    )
    OUT = nc.dram_tensor("out", [128, 1], F32, kind="ExternalOutput")

    nsh = len(SHIFTS)
    nchain = (nsh + CHAIN - 1) // CHAIN
    row = IMGS * WP  # elements per stored row q
    with TileContext(nc) as tc:
        with (
            tc.tile_pool(name="ake", bufs=4) as ake_pool,
            tc.tile_pool(name="ako", bufs=3) as ako_pool,
            tc.tile_pool(name="sc", bufs=1) as sc_pool,
            tc.tile_pool(name="acc", bufs=1) as acc_pool,
        ):
            stage = acc_pool.tile([128, NBLK * nchain], F32)
            scratch = sc_pool.tile([128, IMGS, W], BF16)
            for r in range(NBLK):
                # ake[p,k,i,c] = X[r, p+k, i, c]; one DMA per block,
                # alternating between the two HWDGE rings.  The odd column
                # phase (ako[c] = ake[c+1], for bf16 4B alignment of odd-l
                # shifts) is derived on-device by a DVE 2x_2P copy instead
                # of shipping it over HBM.
                ake = ake_pool.tile([128, 4, IMGS, WP], BF16, tag="ake")
                ako = ako_pool.tile([128, 4, IMGS, WP], BF16, tag="ako")
                eng = nc.sync if r % 2 == 0 else nc.scalar
                eng.dma_start(
                    out=ake[:],
                    in_=bass.AP(
                        X,
                        r * ROWS_BLK * row,
                        [[row, 128], [1, 4 * row]],
                    ),
                )
                flat_e = ake[:].rearrange("p a b c -> p (a b c)")
                flat_o = ako[:].rearrange("p a b c -> p (a b c)")
                ncols = 4 * IMGS * WP
                nc.vector.tensor_copy(
                    out=flat_o[:, 0 : ncols - 2],
                    in_=flat_e[:, 1 : ncols - 1],
                )
                base = ake[:, 0, :, BASE : BASE + W]

                def shifted_ap(si):
                    k, l = SHIFTS[si]
                    if l % 2 == 0:
                        return ake[:, k, :, BASE + l : BASE + l + W]
                    return ako[:, k, :, BASE + l - 1 : BASE + l - 1 + W]

                for ci in range(nchain):
                    grp = list(range(ci * CHAIN, min((ci + 1) * CHAIN, nsh)))
                    col = r * nchain + ci
                    # the chain must run contiguously on DVE: the blk3
                    # accumulator flop carries state between instructions.
                    with tc.tile_critical():
                        for j, si in enumerate(grp):
                            bi = nc.vector._custom_dve(
                                seed_op if j == 0 else cont_op,
                                out=scratch[:],
                                in0=base,
                                in1=shifted_ap(si),
                            )
                            bi.ins.perf_max = 1
                        nc.vector._custom_dve(
                            read_op,
                            out=stage[:, col : col + 1],
                            in0=scratch[:, 0:1, 0:1],
                        )
            part = acc_pool.tile([128, 1], F32)
            nc.vector.tensor_reduce(
                out=part[:],
                in_=stage[:],
                axis=mybir.AxisListType.X,
                op=mybir.AluOpType.add,
            )
            nc.sync.dma_start(out=OUT[:], in_=part[:])
    return nc


_NC = None


def _get_nc():
    global _NC
    if _NC is None:
        _NC = _build_nc()
        if not _NC.is_finalized():
            _NC.finalize()
    return _NC


def _prep_shards(x: np.ndarray) -> list[dict[str, np.ndarray]]:
    """bf16-cast, circular pad, and blockify into the per-core
    (NBLK, 131, IMGS, WP) device layout (even phase only; the odd phase
    is derived on-device)."""
    imgs = np.ascontiguousarray(x.reshape(B * C, H, W), dtype=np.float32)

    def to_bf16(a32):
        b = a32.view(np.uint32)
        return ((b + 0x7FFF + ((b >> 16) & 1)) >> 16).astype(np.uint16)

    imgs_b = to_bf16(imgs)  # (24, H, W) uint16 view of bf16
    HPAD = H + 3
    even = np.zeros((B * C, HPAD, WP), dtype=np.uint16)
    even[:, :H, BASE : BASE + W] = imgs_b
    even[:, :H, :BASE] = imgs_b[:, :, W - BASE :]
    even[:, :H, BASE + W : BASE + W + 3] = imgs_b[:, :, :3]
    even[:, H:, :] = even[:, :3, :]

    shards = even.reshape(NCORES, IMGS, HPAD, WP)
    out = []
    for i in range(NCORES):
        t = shards[i].transpose(1, 0, 2)  # (HPAD, IMGS, WP)
        blk = np.empty((NBLK, ROWS_BLK, IMGS, WP), dtype=np.uint16)
        for r in range(NBLK):
            blk[r] = t[r * RB : r * RB + ROWS_BLK]
        out.append({"x": blk})
    return out


def _run(x: np.ndarray, trace: bool = False):
    import ml_dtypes

    nc = _get_nc()
    in_maps = _prep_shards(x)
    in_maps = [{"x": m["x"].view(ml_dtypes.bfloat16)} for m in in_maps]
    res = run_bass_kernel_spmd(
        nc, in_maps, core_ids=list(range(NCORES)), trace=trace
    )
    total = 0.0
    for r in res.results:
        total += r["out"].astype(np.float64).sum()
    val = WEIGHT * 2.0 * total / float(B * C * H * W)
    return np.float32(val), res


def kernel(x: np.ndarray) -> np.ndarray:
    x = np.asarray(x, dtype=np.float32)
    val, _ = _run(x, trace=False)
    return val



# revision 3
# speedup vs baseline: 1.0626x; 1.0626x over previous
"""BTV loss kernel for Trainium2 (8 NeuronCores, Bass/Tile).

reference: total = sum over 7x7 neighborhood shifts (k,l) != (0,0) of
           sqrt((x - roll(x,(k,l),axis=(2,3)))**2 + 1e-6).sum()
           out = 0.1 * total / x.size

Math: circular-shift symmetry halves the 48 shifts to 24 (doubled at
the end); sqrt(d^2+1e-6) ~= |d| (rel err ~3e-6); bf16 inputs add ~1e-5.

Layout (host-prepared, "band-major"): partition p holds rows
8p..8p+10 of each image (8 data bands + 3 halo bands), columns padded
circularly [4 left | 1024 | 3 right | 1 pad] = 1032.  A vertical shift
k is a band offset, a horizontal shift l a column offset -- both plain
AP offsets, so one DVE instruction covers a whole (image, shift) unit
of 128x8x1024 elements.  An odd-phase copy (cols shifted by one)
keeps odd-l operands 4B-aligned for DVE 2x mode.

Engines (work units = 24 shifts x 3 images = 72 per core):
  - DVE: custom fused |a-b|+accumulate op at 2x (~0.53 ns/elem),
    one op per unit, chained via the blk3 accumulator flop.
  - PE+ACT: remaining units as 512-col chunks: psum = I@base + (-I)@
    shifted (two matmuls), ACT drains Abs(psum) with accum_out.
  - Final: DVE reduces the f32 stage; host sums partials in f64.

Distribution: data parallel over the 24 (b,c) images, 3 per core.
"""

import dataclasses
import re
from operator import add as _py_add

import numpy as np

import concourse.bass as bass
import concourse.bacc as bacc_mod
import concourse.mybir as mybir
from concourse import dve_ops as _dvo
from concourse.dve_spec import AluOp as _DveAluOp
from concourse.dve_spec import Bin, Spec, Src0, Src1
from concourse.tile import TileContext
from concourse.bass_utils import run_bass_kernel_spmd

B, C, H, W = 8, 3, 1024, 1024
NCORES = 8
IMGS = (B * C) // NCORES        # images per core = 3
P = 128                         # partitions
BANDS = 11                      # 8 data bands + 3 halo
BASE = 4                        # left col pad (even => 4B-aligned bf16)
WPAD = W + BASE + 3 + 1         # 1032
SHIFTS = [(k, l) for k in range(0, 4) for l in range(-3, 4) if (k > 0 or l > 0)]
assert len(SHIFTS) == 24

# (shift, img) units on the PE+ACT lane; the rest run on DVE.
PE_SHIFTS = [(1, 3), (1, -3), (2, 3), (2, -3), (3, 3), (3, -3), (0, 3)]
PE_EXTRA = [((0, 1), 0), ((0, 1), 1)]  # partial shift for load balance
CHUNK = 512
GRP = 4                         # psum chunks per ACT drain (4 banks)

WEIGHT = 0.1
F32 = mybir.dt.float32
BF16 = mybir.dt.bfloat16

_OPS_CACHE = None


def _mk_absdiff_uop(two_x: bool, kind: str):
    """One UopConfig for the absdiff-accumulate family.

    kind: "seed"   - first element(-pair) loads the accumulator (blk3)
          "steady" - accumulates into blk3's CURR_ALU_OUT flop
    """
    from concourse.dve_uop import (
        ENABLE,
        AluInp,
        DelayInp,
        InpSel,
        OutPath,
        OutSel,
        Trigger,
        UopConfig,
    )
    from concourse.dve_uop import AluOp as UAluOp

    seed = kind == "seed"
    u = UopConfig()
    u.enable_input(InpSel.SRC_0, 0)
    u.enable_input(InpSel.SRC_1, 1)
    if two_x:
        u.enable_input(InpSel.SRC_0_HI, 2)
        u.enable_input(InpSel.SRC_1_HI, 3)
    u.accum_enabled = ENABLE
    dp = u.datapath_config
    dp[0].enable_alu(UAluOp.ABSOLUTE_DIFF, AluInp.PREV_ALU_OUT, AluInp.PREV_DELAY_0)
    if two_x:
        dp[0].pass_through_delay(1, 2)
        dp[1].enable_alu(
            UAluOp.ABSOLUTE_DIFF, AluInp.PREV_DELAY_1, AluInp.PREV_DELAY_2
        )
        dp[1].enable_delay_from_src(DelayInp.PREV_ALU_OUT, 0)
        dp[2].enable_alu(UAluOp.ADD, AluInp.PREV_ALU_OUT, AluInp.PREV_DELAY_0)
        dp[2].enable_delay_from_src(DelayInp.PREV_ALU_OUT, 1)
        dp[2].pass_through_delay(0)
    else:
        dp[1].enable_alu(UAluOp.BYPASS, AluInp.PREV_ALU_OUT, AluInp.PREV_ALU_OUT)
        dp[1].enable_delay_from_src(DelayInp.PREV_ALU_OUT, 0)
        dp[2].enable_alu(UAluOp.BYPASS, AluInp.PREV_ALU_OUT, AluInp.PREV_ALU_OUT)
        dp[2].pass_through_delay(0)
    if seed:
        dp[3].enable_alu(UAluOp.BYPASS, AluInp.PREV_ALU_OUT, AluInp.PREV_ALU_OUT)
    else:
        dp[3].enable_alu(UAluOp.ADD, AluInp.CURR_ALU_OUT, AluInp.PREV_ALU_OUT)
    dp[3].pass_through_delay(0, 1) if two_x else dp[3].pass_through_delay(0)
    dp[3].alu_out_a_enable = ENABLE
    for b in (4, 5, 6, 7):
        dp[b].enable_alu(UAluOp.BYPASS, AluInp.PREV_ALU_OUT, AluInp.PREV_ALU_OUT)
        dp[b].pass_through_delay(0, 1) if two_x else dp[b].pass_through_delay(0)
        dp[b].alu_out_a_enable = ENABLE
    u.require_inp0 = ENABLE
    u.require_inp1 = ENABLE
    u.enable_output(OutSel.DELAY_0, OutPath.WR0_LO)
    if two_x:
        u.enable_output(OutSel.DELAY_1, OutPath.WR0_HI)
    if seed:
        u.trigger = (Trigger.COUNT, Trigger.SRC_TENSOR_DONE, Trigger.NONE)
        u.next_uop = (1, 0, 0)
        u.repeat_count = 1
    else:
        u.trigger = (Trigger.SRC_TENSOR_DONE, Trigger.NONE, Trigger.NONE)
        u.next_uop = (0, 0, 0)
    return u


def _mk_read_uop():
    """Route blk3's persistent accumulator flop to WR0_LO."""
    from concourse.dve_uop import (
        ENABLE,
        AluInp,
        InpSel,
        OutPath,
        OutSel,
        Trigger,
        UopConfig,
    )
    from concourse.dve_uop import AluOp as UAluOp

    u = UopConfig()
    u.enable_input(InpSel.SRC_0, 0)
    dp = u.datapath_config
    for b in (0, 1, 2):
        dp[b].enable_alu(UAluOp.BYPASS, AluInp.PREV_ALU_OUT, AluInp.PREV_ALU_OUT)
    dp[3].enable_alu(UAluOp.BYPASS, AluInp.CURR_ALU_OUT, AluInp.CURR_ALU_OUT)
    for b in (4, 5, 6, 7):
        dp[b].enable_alu(UAluOp.BYPASS, AluInp.PREV_ALU_OUT, AluInp.PREV_ALU_OUT)
    u.require_inp0 = ENABLE
    u.enable_output(OutSel.ALU_OUT, OutPath.WR0_LO)
    u.trigger = (Trigger.SRC_TENSOR_DONE, Trigger.NONE, Trigger.NONE)
    u.next_uop = (0, 0, 0)
    return u


class _HandDveOp(_dvo.DveOp):
    """DveOp with hand-authored uop programs (1x and optional 2x)."""

    BUILDERS = {}  # name -> (build_1x, build_2x_or_None, rd1_en)

    def compile(self, ver):
        from concourse.dve_uop import DveOpSpec

        key = (self.name, ver)
        if (r := _dvo._COMPILE_CACHE.get(key)) is not None:
            return r
        b1, b2, rd1 = self.BUILDERS[self.name]
        result = DveOpSpec(
            name=self.name,
            opcode=_dvo.get_dve_sub_opcode(self.name),
            uops=b1(),
            uops_2x=(b2() if b2 is not None else None),
            rd1_en=rd1,
        )
        got = result.sha(ver)
        if self.uops_sha.get(ver) != got:
            raise ValueError(f"sha drift ({ver}: {got} != pinned)")
        _dvo._COMPILE_CACHE[key] = result
        return result


def _register(name, spec, build_1x, build_2x, rd1_en):
    _HandDveOp.BUILDERS[name] = (build_1x, build_2x, rd1_en)
    op = _HandDveOp(name, spec, subdim=False, uops_sha={})
    _dvo._SUB_OPCODE_FOR_NAME[name] = _dvo._CUSTOM_DVE_ROW_BASE + len(_dvo.OPS)
    shas = {}
    for ver in ("v3", "v4"):
        try:
            op.compile(ver)
            shas[ver] = op.uops_sha.get(ver)
        except ValueError as e:
            m = re.search(r"([0-9a-f]{16})", str(e))
            if not m:
                raise
            shas[ver] = m.group(1)
    op = dataclasses.replace(op, uops_sha=shas)
    _dvo.OPS.append(op)
    _dvo.CUSTOM_DVE_SPECS[name] = spec
    return op


def _get_ops():
    """Register (once per process) the custom DVE ops; return
    (seed, cont, read)."""
    global _OPS_CACHE
    if _OPS_CACHE is not None:
        return _OPS_CACHE
    have = {op.name: op for op in _dvo.OPS}
    if "ABSDIFF_ACC_SEED_ANT" in have:
        _OPS_CACHE = (
            have["ABSDIFF_ACC_SEED_ANT"],
            have["ABSDIFF_ACC_CONT_ANT"],
            have["ABSDIFF_ACC_READ_ANT"],
        )
        return _OPS_CACHE

    def _ref_acc(in0, in1, s0, s1, imm2):
        b = np.abs(in0.astype(np.float32) - in1.astype(np.float32)).astype(
            np.float32
        )
        return b, b.reshape(b.shape[0], -1).sum(axis=-1, keepdims=True)

    spec_acc = Spec(
        body=Bin(_DveAluOp.ABSOLUTE_DIFF, Src0, Src1),
        accum=_py_add,
        reference=_ref_acc,
    )
    spec_read = Spec(
        body=Src0,
        reference=lambda in0, in1, s0, s1, imm2: in0.astype(np.float32),
    )
    seed = _register(
        "ABSDIFF_ACC_SEED_ANT",
        spec_acc,
        lambda: [_mk_absdiff_uop(False, "seed"), _mk_absdiff_uop(False, "steady")],
        lambda: [_mk_absdiff_uop(True, "seed"), _mk_absdiff_uop(True, "steady")],
        True,
    )
    cont = _register(
        "ABSDIFF_ACC_CONT_ANT",
        spec_acc,
        lambda: [_mk_absdiff_uop(False, "steady")],
        lambda: [_mk_absdiff_uop(True, "steady")],
        True,
    )
    read = _register(
        "ABSDIFF_ACC_READ_ANT",
        spec_read,
        lambda: [_mk_read_uop()],
        None,
        False,
    )
    _OPS_CACHE = (seed, cont, read)
    return _OPS_CACHE


def _unit_assignment():
    """Return (dve_units, pe_units): lists of (shift_idx, img),
    image-major ordered."""
    pe = set()
    for s in PE_SHIFTS:
        si = SHIFTS.index(s)
        for i in range(IMGS):
            pe.add((si, i))
    for s, i in PE_EXTRA:
        pe.add((SHIFTS.index(s), i))
    dve, peu = [], []
    for i in range(IMGS):
        for si in range(len(SHIFTS)):
            u = (si, i)
            (peu if u in pe else dve).append(u)
    return dve, peu


def _build_nc():
    seed_op, cont_op, read_op = _get_ops()
    dve_units, pe_units = _unit_assignment()
    n_drains = len(pe_units) * (8 * (W // CHUNK) // GRP)
    nstage = IMGS + n_drains  # 1 col per DVE image-chain + 1 per drain

    nc = bacc_mod.Bacc("TRN2", target_bir_lowering=False)
    X = nc.dram_tensor(
        "x", [IMGS, 2, P, BANDS, WPAD], BF16, kind="ExternalInput"
    )
    WT = nc.dram_tensor("w", [P, 2 * P], BF16, kind="ExternalInput")
    OUT = nc.dram_tensor("out", [P, 1], F32, kind="ExternalOutput")

    with TileContext(nc) as tc:
        with (
            tc.tile_pool(name="data", bufs=1) as data_pool,
            tc.tile_pool(name="sc", bufs=1) as sc_pool,
            tc.tile_pool(name="acts", bufs=2) as acts_pool,
            tc.tile_pool(name="ps", bufs=2, space="PSUM") as ps_pool,
        ):
            wt = data_pool.tile([P, 2 * P], BF16)
            nc.sync.dma_start(out=wt[:], in_=WT[:])
            e = [
                data_pool.tile([P, BANDS, WPAD], BF16, name=f"e{i}")
                for i in range(IMGS)
            ]
            o = [
                data_pool.tile([P, BANDS, WPAD], BF16, name=f"o{i}")
                for i in range(IMGS)
            ]
            for i in range(IMGS):
                eng = nc.sync if i % 2 == 0 else nc.gpsimd
                eng.dma_start(out=e[i][:], in_=X[i, 0])
                eng2 = nc.gpsimd if i % 2 == 0 else nc.sync
                eng2.dma_start(out=o[i][:], in_=X[i, 1])
            scratch = sc_pool.tile([P, 8 * W], BF16)
            stage = sc_pool.tile([P, nstage], F32)
            wI = wt[:, 0:P]
            wnI = wt[:, P : 2 * P]

            def in1_ap(i, k, l, c0, nb=8, b0=0, width=None):
                wd = CHUNK if width is None else width
                if l % 2 == 0:
                    return e[i][:, b0 + k : b0 + k + nb, BASE + l + c0 : BASE + l + c0 + wd]
                return o[i][:, b0 + k : b0 + k + nb, BASE + l - 1 + c0 : BASE + l - 1 + c0 + wd]

            # ---- DVE lane: one chain per image ----
            col = 0
            by_img = {}
            for si, i in dve_units:
                by_img.setdefault(i, []).append(si)
            dve_chains = []
            for i in sorted(by_img):
                dve_chains.append((i, by_img[i], col))
                col += 1

            # ---- emit: interleave by image for early start ----
            for i, sis, scol in dve_chains:
                with tc.tile_critical():
                    for j, si in enumerate(sis):
                        k, l = SHIFTS[si]
                        bi = nc.vector._custom_dve(
                            seed_op if j == 0 else cont_op,
                            out=scratch[:],
                            in0=e[i][:, 0:8, BASE : BASE + W],
                            in1=in1_ap(i, k, l, 0, width=W),
                        )
                        bi.ins.perf_max = 1
                    nc.vector._custom_dve(
                        read_op,
                        out=stage[:, scol : scol + 1],
                        in0=scratch[:, 0:1],
                    )

            # ---- PE + ACT lane ----
            for ui, (si, i) in enumerate(pe_units):
                k, l = SHIFTS[si]
                # 16 chunks = 8 bands x 2 col-chunks; groups of 4
                for g in range(4):
                    psum = ps_pool.tile([P, GRP * CHUNK], F32, tag="ps")
                    for c in range(GRP):
                        chunk = g * GRP + c
                        b = chunk % 8
                        c0 = (chunk // 8) * CHUNK
                        nc.tensor.matmul(
                            out=psum[:, c * CHUNK : (c + 1) * CHUNK],
                            lhsT=wI,
                            rhs=e[i][:, b, BASE + c0 : BASE + c0 + CHUNK],
                            start=True,
                            stop=False,
                        )
                        nc.tensor.matmul(
                            out=psum[:, c * CHUNK : (c + 1) * CHUNK],
                            lhsT=wnI,
                            rhs=in1_ap(i, k, l, c0, nb=1, b0=b)[:, 0, :],
                            start=False,
                            stop=True,
                        )
                    asc = acts_pool.tile([P, GRP * CHUNK], BF16, tag="asc")
                    nc.scalar.activation(
                        out=asc[:],
                        in_=psum[:],
                        func=mybir.ActivationFunctionType.Abs,
                        accum_out=stage[:, col : col + 1],
                    )
                    col += 1

            part = sc_pool.tile([P, 1], F32)
            nc.vector.tensor_reduce(
                out=part[:],
                in_=stage[:],
                axis=mybir.AxisListType.X,
                op=mybir.AluOpType.add,
            )
            nc.sync.dma_start(out=OUT[:], in_=part[:])
    return nc


_NC = None


def _get_nc():
    global _NC
    if _NC is None:
        _NC = _build_nc()
        if not _NC.is_finalized():
            _NC.finalize()
    return _NC


def _to_bf16(a32: np.ndarray) -> np.ndarray:
    b = np.ascontiguousarray(a32, dtype=np.float32).view(np.uint32)
    return ((b + 0x7FFF + ((b >> 16) & 1)) >> 16).astype(np.uint16)


def _prep_shards(x: np.ndarray) -> list[dict[str, np.ndarray]]:
    """bf16-cast and pack into the per-core band-major layout
    [IMGS, 2 phases, 128, BANDS, WPAD] (uint16 views of bf16)."""
    import ml_dtypes

    imgs = _to_bf16(x.reshape(B * C, H, W))  # (24, 1024, 1024) u16
    # circular column pad: [W-4..W-1][0..W-1][0..2][0]
    pad = np.zeros((B * C, H, WPAD), dtype=np.uint16)
    pad[:, :, BASE : BASE + W] = imgs
    pad[:, :, :BASE] = imgs[:, :, W - BASE :]
    pad[:, :, BASE + W : BASE + W + 3] = imgs[:, :, :3]
    # bands: E[p, b] = row (8p + b) % H
    rows = (8 * np.arange(P)[:, None] + np.arange(BANDS)[None, :]) % H
    even = pad[:, rows, :]  # (24, 128, 11, 1032)
    odd = np.zeros_like(even)
    odd[..., : WPAD - 1] = even[..., 1:]

    wk = np.zeros((P, 2 * P), dtype=np.float32)
    wk[:, 0:P] = np.eye(P)
    wk[:, P : 2 * P] = -np.eye(P)
    wv = wk.astype(ml_dtypes.bfloat16)

    out = []
    for ci in range(NCORES):
        xs = np.stack(
            [
                np.stack([even[ci * IMGS + i], odd[ci * IMGS + i]])
                for i in range(IMGS)
            ]
        )  # (IMGS, 2, 128, 11, 1032) u16
        out.append({"x": xs.view(ml_dtypes.bfloat16), "w": wv})
    return out


def _run(x: np.ndarray, trace: bool = False):
    nc = _get_nc()
    in_maps = _prep_shards(x)
    res = run_bass_kernel_spmd(
        nc, in_maps, core_ids=list(range(NCORES)), trace=trace
    )
    total = 0.0
    for r in res.results:
        total += r["out"].astype(np.float64).sum()
    val = WEIGHT * 2.0 * total / float(B * C * H * W)
    return np.float32(val), res


def kernel(x: np.ndarray) -> np.ndarray:
    x = np.asarray(x, dtype=np.float32)
    val, _ = _run(x, trace=False)
    return val


# revision 4
# speedup vs baseline: 1.9678x; 1.8518x over previous
"""BTV loss kernel for Trainium2 (8 NeuronCores, Bass/Tile).

reference: total = sum over 7x7 neighborhood shifts (k,l) != (0,0) of
           sqrt((x - roll(x,(k,l),axis=(2,3)))**2 + 1e-6).sum()
           out = 0.1 * total / x.size

Math: circular-shift symmetry halves the 48 shifts to 24 (doubled at
the end); sqrt(d^2+1e-6) ~= |d| (rel err ~3e-6); bf16 inputs add ~1e-5.

Layout (host-prepared, "band-major"): partition p holds rows
8p..8p+10 of each image (8 data bands + 3 halo bands), columns padded
circularly [4 left | 1024 | 3 right | 1 pad] = 1032.  A vertical shift
k is a band offset, a horizontal shift l a column offset -- both plain
AP offsets, so one DVE instruction covers a whole (image, shift) unit
of 128x8x1024 elements.  An odd-phase copy (cols shifted by one)
keeps odd-l operands 4B-aligned for DVE 2x mode.

Engines (work units = 24 shifts x 3 images = 72 per core):
  - DVE: custom fused |a-b|+accumulate op at 2x (~0.53 ns/elem),
    one op per unit, chained via the blk3 accumulator flop.
  - PE+ACT: remaining units as 512-col chunks: psum = I@base + (-I)@
    shifted (two matmuls), ACT drains Abs(psum) with accum_out.
  - Final: DVE reduces the f32 stage; host sums partials in f64.

Distribution: data parallel over the 24 (b,c) images, 3 per core.
"""

import dataclasses
import re
from operator import add as _py_add

import numpy as np

import concourse.bass as bass
import concourse.bacc as bacc_mod
import concourse.mybir as mybir
from concourse import dve_ops as _dvo
from concourse.dve_spec import AluOp as _DveAluOp
from concourse.dve_spec import Bin, Spec, Src0, Src1
from concourse.tile import TileContext
from concourse.bass_utils import run_bass_kernel_spmd

B, C, H, W = 8, 3, 1024, 1024
NCORES = 8
IMGS = (B * C) // NCORES        # images per core = 3
P = 128                         # partitions
BANDS = 11                      # 8 data bands + 3 halo
BASE = 4                        # left col pad (even => 4B-aligned bf16)
WPAD = W + BASE + 3 + 1         # 1032
SHIFTS = [(k, l) for k in range(0, 4) for l in range(-3, 4) if (k > 0 or l > 0)]
assert len(SHIFTS) == 24

# (shift, img) units on the PE+ACT lane; the rest run on DVE.
PE_SHIFTS = [(1, 3), (1, -3), (2, 3), (2, -3), (3, 3), (3, -3), (0, 3)]
PE_EXTRA = [((0, 1), 0), ((0, 1), 1)]  # partial shift for load balance
CHUNK = 512
GRP = 4                         # psum chunks per ACT drain (4 banks)

WEIGHT = 0.1
F32 = mybir.dt.float32
BF16 = mybir.dt.bfloat16

_OPS_CACHE = None


def _mk_absdiff_uop(two_x: bool, kind: str):
    """One UopConfig for the absdiff-accumulate family.

    kind: "seed"   - first element(-pair) loads the accumulator (blk3)
          "steady" - accumulates into blk3's CURR_ALU_OUT flop
    """
    from concourse.dve_uop import (
        ENABLE,
        AluInp,
        DelayInp,
        InpSel,
        OutPath,
        OutSel,
        Trigger,
        UopConfig,
    )
    from concourse.dve_uop import AluOp as UAluOp

    seed = kind == "seed"
    u = UopConfig()
    u.enable_input(InpSel.SRC_0, 0)
    u.enable_input(InpSel.SRC_1, 1)
    if two_x:
        u.enable_input(InpSel.SRC_0_HI, 2)
        u.enable_input(InpSel.SRC_1_HI, 3)
    u.accum_enabled = ENABLE
    dp = u.datapath_config
    dp[0].enable_alu(UAluOp.ABSOLUTE_DIFF, AluInp.PREV_ALU_OUT, AluInp.PREV_DELAY_0)
    if two_x:
        dp[0].pass_through_delay(1, 2)
        dp[1].enable_alu(
            UAluOp.ABSOLUTE_DIFF, AluInp.PREV_DELAY_1, AluInp.PREV_DELAY_2
        )
        dp[1].enable_delay_from_src(DelayInp.PREV_ALU_OUT, 0)
        dp[2].enable_alu(UAluOp.ADD, AluInp.PREV_ALU_OUT, AluInp.PREV_DELAY_0)
        dp[2].enable_delay_from_src(DelayInp.PREV_ALU_OUT, 1)
        dp[2].pass_through_delay(0)
    else:
        dp[1].enable_alu(UAluOp.BYPASS, AluInp.PREV_ALU_OUT, AluInp.PREV_ALU_OUT)
        dp[1].enable_delay_from_src(DelayInp.PREV_ALU_OUT, 0)
        dp[2].enable_alu(UAluOp.BYPASS, AluInp.PREV_ALU_OUT, AluInp.PREV_ALU_OUT)
        dp[2].pass_through_delay(0)
    if seed:
        dp[3].enable_alu(UAluOp.BYPASS, AluInp.PREV_ALU_OUT, AluInp.PREV_ALU_OUT)
    else:
        dp[3].enable_alu(UAluOp.ADD, AluInp.CURR_ALU_OUT, AluInp.PREV_ALU_OUT)
    dp[3].pass_through_delay(0, 1) if two_x else dp[3].pass_through_delay(0)
    dp[3].alu_out_a_enable = ENABLE
    for b in (4, 5, 6, 7):
        dp[b].enable_alu(UAluOp.BYPASS, AluInp.PREV_ALU_OUT, AluInp.PREV_ALU_OUT)
        dp[b].pass_through_delay(0, 1) if two_x else dp[b].pass_through_delay(0)
        dp[b].alu_out_a_enable = ENABLE
    u.require_inp0 = ENABLE
    u.require_inp1 = ENABLE
    u.enable_output(OutSel.DELAY_0, OutPath.WR0_LO)
    if two_x:
        u.enable_output(OutSel.DELAY_1, OutPath.WR0_HI)
    if seed:
        u.trigger = (Trigger.COUNT, Trigger.SRC_TENSOR_DONE, Trigger.NONE)
        u.next_uop = (1, 0, 0)
        u.repeat_count = 1
    else:
        u.trigger = (Trigger.SRC_TENSOR_DONE, Trigger.NONE, Trigger.NONE)
        u.next_uop = (0, 0, 0)
    return u


def _mk_read_uop():
    """Route blk3's persistent accumulator flop to WR0_LO."""
    from concourse.dve_uop import (
        ENABLE,
        AluInp,
        InpSel,
        OutPath,
        OutSel,
        Trigger,
        UopConfig,
    )
    from concourse.dve_uop import AluOp as UAluOp

    u = UopConfig()
    u.enable_input(InpSel.SRC_0, 0)
    dp = u.datapath_config
    for b in (0, 1, 2):
        dp[b].enable_alu(UAluOp.BYPASS, AluInp.PREV_ALU_OUT, AluInp.PREV_ALU_OUT)
    dp[3].enable_alu(UAluOp.BYPASS, AluInp.CURR_ALU_OUT, AluInp.CURR_ALU_OUT)
    for b in (4, 5, 6, 7):
        dp[b].enable_alu(UAluOp.BYPASS, AluInp.PREV_ALU_OUT, AluInp.PREV_ALU_OUT)
    u.require_inp0 = ENABLE
    u.enable_output(OutSel.ALU_OUT, OutPath.WR0_LO)
    u.trigger = (Trigger.SRC_TENSOR_DONE, Trigger.NONE, Trigger.NONE)
    u.next_uop = (0, 0, 0)
    return u


class _HandDveOp(_dvo.DveOp):
    """DveOp with hand-authored uop programs (1x and optional 2x)."""

    BUILDERS = {}  # name -> (build_1x, build_2x_or_None, rd1_en)

    def compile(self, ver):
        from concourse.dve_uop import DveOpSpec

        key = (self.name, ver)
        if (r := _dvo._COMPILE_CACHE.get(key)) is not None:
            return r
        b1, b2, rd1 = self.BUILDERS[self.name]
        result = DveOpSpec(
            name=self.name,
            opcode=_dvo.get_dve_sub_opcode(self.name),
            uops=b1(),
            uops_2x=(b2() if b2 is not None else None),
            rd1_en=rd1,
        )
        got = result.sha(ver)
        if self.uops_sha.get(ver) != got:
            raise ValueError(f"sha drift ({ver}: {got} != pinned)")
        _dvo._COMPILE_CACHE[key] = result
        return result


def _register(name, spec, build_1x, build_2x, rd1_en):
    _HandDveOp.BUILDERS[name] = (build_1x, build_2x, rd1_en)
    op = _HandDveOp(name, spec, subdim=False, uops_sha={})
    _dvo._SUB_OPCODE_FOR_NAME[name] = _dvo._CUSTOM_DVE_ROW_BASE + len(_dvo.OPS)
    shas = {}
    for ver in ("v3", "v4"):
        try:
            op.compile(ver)
            shas[ver] = op.uops_sha.get(ver)
        except ValueError as e:
            m = re.search(r"([0-9a-f]{16})", str(e))
            if not m:
                raise
            shas[ver] = m.group(1)
    op = dataclasses.replace(op, uops_sha=shas)
    _dvo.OPS.append(op)
    _dvo.CUSTOM_DVE_SPECS[name] = spec
    return op


def _get_ops():
    """Register (once per process) the custom DVE ops; return
    (seed, cont, read)."""
    global _OPS_CACHE
    if _OPS_CACHE is not None:
        return _OPS_CACHE
    have = {op.name: op for op in _dvo.OPS}
    if "ABSDIFF_ACC_SEED_ANT" in have:
        _OPS_CACHE = (
            have["ABSDIFF_ACC_SEED_ANT"],
            have["ABSDIFF_ACC_CONT_ANT"],
            have["ABSDIFF_ACC_READ_ANT"],
        )
        return _OPS_CACHE

    def _ref_acc(in0, in1, s0, s1, imm2):
        b = np.abs(in0.astype(np.float32) - in1.astype(np.float32)).astype(
            np.float32
        )
        return b, b.reshape(b.shape[0], -1).sum(axis=-1, keepdims=True)

    spec_acc = Spec(
        body=Bin(_DveAluOp.ABSOLUTE_DIFF, Src0, Src1),
        accum=_py_add,
        reference=_ref_acc,
    )
    spec_read = Spec(
        body=Src0,
        reference=lambda in0, in1, s0, s1, imm2: in0.astype(np.float32),
    )
    seed = _register(
        "ABSDIFF_ACC_SEED_ANT",
        spec_acc,
        lambda: [_mk_absdiff_uop(False, "seed"), _mk_absdiff_uop(False, "steady")],
        lambda: [_mk_absdiff_uop(True, "seed"), _mk_absdiff_uop(True, "steady")],
        True,
    )
    cont = _register(
        "ABSDIFF_ACC_CONT_ANT",
        spec_acc,
        lambda: [_mk_absdiff_uop(False, "steady")],
        lambda: [_mk_absdiff_uop(True, "steady")],
        True,
    )
    read = _register(
        "ABSDIFF_ACC_READ_ANT",
        spec_read,
        lambda: [_mk_read_uop()],
        None,
        False,
    )
    _OPS_CACHE = (seed, cont, read)
    return _OPS_CACHE


def _unit_assignment():
    """Return (dve_units, pe_units): lists of (shift_idx, img),
    image-major ordered."""
    pe = set()
    for s in PE_SHIFTS:
        si = SHIFTS.index(s)
        for i in range(IMGS):
            pe.add((si, i))
    for s, i in PE_EXTRA:
        pe.add((SHIFTS.index(s), i))
    dve, peu = [], []
    for i in range(IMGS):
        for si in range(len(SHIFTS)):
            u = (si, i)
            (peu if u in pe else dve).append(u)
    return dve, peu


def _build_nc():
    seed_op, cont_op, read_op = _get_ops()
    dve_units, pe_units = _unit_assignment()
    n_drains = len(pe_units) * (8 * (W // CHUNK) // GRP)
    nstage = IMGS + n_drains  # 1 col per DVE image-chain + 1 per drain

    nc = bacc_mod.Bacc("TRN2", target_bir_lowering=False)
    X = nc.dram_tensor(
        "x", [IMGS, 2, P, BANDS, WPAD], BF16, kind="ExternalInput"
    )
    WT = nc.dram_tensor("w", [P, 2 * P], BF16, kind="ExternalInput")
    OUT = nc.dram_tensor("out", [P, 1], F32, kind="ExternalOutput")

    with TileContext(nc) as tc:
        with (
            tc.tile_pool(name="data", bufs=1) as data_pool,
            tc.tile_pool(name="sc", bufs=1) as sc_pool,
            tc.tile_pool(name="acts", bufs=2) as acts_pool,
            tc.tile_pool(name="ps", bufs=2, space="PSUM") as ps_pool,
        ):
            wt = data_pool.tile([P, 2 * P], BF16)
            nc.sync.dma_start(out=wt[:], in_=WT[:])
            e = [
                data_pool.tile([P, BANDS, WPAD], BF16, name=f"e{i}")
                for i in range(IMGS)
            ]
            o = [
                data_pool.tile([P, BANDS, WPAD], BF16, name=f"o{i}")
                for i in range(IMGS)
            ]
            for i in range(IMGS):
                eng = nc.sync if i % 2 == 0 else nc.gpsimd
                eng.dma_start(out=e[i][:], in_=X[i, 0])
                eng2 = nc.gpsimd if i % 2 == 0 else nc.sync
                eng2.dma_start(out=o[i][:], in_=X[i, 1])
            scratch = sc_pool.tile([P, 8 * W], BF16)
            stage = sc_pool.tile([P, nstage], F32)
            wI = wt[:, 0:P]
            wnI = wt[:, P : 2 * P]

            def in1_ap(i, k, l, c0, nb=8, b0=0, width=None):
                wd = CHUNK if width is None else width
                if l % 2 == 0:
                    return e[i][:, b0 + k : b0 + k + nb, BASE + l + c0 : BASE + l + c0 + wd]
                return o[i][:, b0 + k : b0 + k + nb, BASE + l - 1 + c0 : BASE + l - 1 + c0 + wd]

            # ---- DVE lane: one chain per image ----
            col = 0
            by_img = {}
            for si, i in dve_units:
                by_img.setdefault(i, []).append(si)
            dve_chains = []
            for i in sorted(by_img):
                dve_chains.append((i, by_img[i], col))
                col += 1

            # ---- emit: interleave by image for early start ----
            # No tile_critical: chain contiguity on DVE is already forced
            # by WAW/WAR hazards on `scratch` (every op writes it, READ
            # reads it), and nothing else runs on the Vector engine.
            # tile_critical would drain ALL engines at each chain end,
            # serializing the PE/ACT lane against the DVE lane.
            for i, sis, scol in dve_chains:
                for j, si in enumerate(sis):
                    k, l = SHIFTS[si]
                    bi = nc.vector._custom_dve(
                        seed_op if j == 0 else cont_op,
                        out=scratch[:],
                        in0=e[i][:, 0:8, BASE : BASE + W],
                        in1=in1_ap(i, k, l, 0, width=W),
                    )
                    bi.ins.perf_max = 1
                nc.vector._custom_dve(
                    read_op,
                    out=stage[:, scol : scol + 1],
                    in0=scratch[:, 0:1],
                )

            # ---- PE + ACT lane ----
            for ui, (si, i) in enumerate(pe_units):
                k, l = SHIFTS[si]
                # 16 chunks = 8 bands x 2 col-chunks; groups of 4
                for g in range(4):
                    psum = ps_pool.tile([P, GRP * CHUNK], F32, tag="ps")
                    for c in range(GRP):
                        chunk = g * GRP + c
                        b = chunk % 8
                        c0 = (chunk // 8) * CHUNK
                        nc.tensor.matmul(
                            out=psum[:, c * CHUNK : (c + 1) * CHUNK],
                            lhsT=wI,
                            rhs=e[i][:, b, BASE + c0 : BASE + c0 + CHUNK],
                            start=True,
                            stop=False,
                        )
                        nc.tensor.matmul(
                            out=psum[:, c * CHUNK : (c + 1) * CHUNK],
                            lhsT=wnI,
                            rhs=in1_ap(i, k, l, c0, nb=1, b0=b)[:, 0, :],
                            start=False,
                            stop=True,
                        )
                    asc = acts_pool.tile([P, GRP * CHUNK], BF16, tag="asc")
                    nc.scalar.activation(
                        out=asc[:],
                        in_=psum[:],
                        func=mybir.ActivationFunctionType.Abs,
                        accum_out=stage[:, col : col + 1],
                    )
                    col += 1

            part = sc_pool.tile([P, 1], F32)
            nc.vector.tensor_reduce(
                out=part[:],
                in_=stage[:],
                axis=mybir.AxisListType.X,
                op=mybir.AluOpType.add,
            )
            nc.sync.dma_start(out=OUT[:], in_=part[:])
    return nc


_NC = None


def _get_nc():
    global _NC
    if _NC is None:
        _NC = _build_nc()
        if not _NC.is_finalized():
            _NC.finalize()
    return _NC


def _to_bf16(a32: np.ndarray) -> np.ndarray:
    b = np.ascontiguousarray(a32, dtype=np.float32).view(np.uint32)
    return ((b + 0x7FFF + ((b >> 16) & 1)) >> 16).astype(np.uint16)


def _prep_shards(x: np.ndarray) -> list[dict[str, np.ndarray]]:
    """bf16-cast and pack into the per-core band-major layout
    [IMGS, 2 phases, 128, BANDS, WPAD] (uint16 views of bf16)."""
    import ml_dtypes

    imgs = _to_bf16(x.reshape(B * C, H, W))  # (24, 1024, 1024) u16
    # circular column pad: [W-4..W-1][0..W-1][0..2][0]
    pad = np.zeros((B * C, H, WPAD), dtype=np.uint16)
    pad[:, :, BASE : BASE + W] = imgs
    pad[:, :, :BASE] = imgs[:, :, W - BASE :]
    pad[:, :, BASE + W : BASE + W + 3] = imgs[:, :, :3]
    # bands: E[p, b] = row (8p + b) % H
    rows = (8 * np.arange(P)[:, None] + np.arange(BANDS)[None, :]) % H
    even = pad[:, rows, :]  # (24, 128, 11, 1032)
    odd = np.zeros_like(even)
    odd[..., : WPAD - 1] = even[..., 1:]

    wk = np.zeros((P, 2 * P), dtype=np.float32)
    wk[:, 0:P] = np.eye(P)
    wk[:, P : 2 * P] = -np.eye(P)
    wv = wk.astype(ml_dtypes.bfloat16)

    out = []
    for ci in range(NCORES):
        xs = np.stack(
            [
                np.stack([even[ci * IMGS + i], odd[ci * IMGS + i]])
                for i in range(IMGS)
            ]
        )  # (IMGS, 2, 128, 11, 1032) u16
        out.append({"x": xs.view(ml_dtypes.bfloat16), "w": wv})
    return out


def _run(x: np.ndarray, trace: bool = False):
    nc = _get_nc()
    in_maps = _prep_shards(x)
    res = run_bass_kernel_spmd(
        nc, in_maps, core_ids=list(range(NCORES)), trace=trace
    )
    total = 0.0
    for r in res.results:
        total += r["out"].astype(np.float64).sum()
    val = WEIGHT * 2.0 * total / float(B * C * H * W)
    return np.float32(val), res


def kernel(x: np.ndarray) -> np.ndarray:
    x = np.asarray(x, dtype=np.float32)
    val, _ = _run(x, trace=False)
    return val
